# revision 18
# baseline (speedup 1.0000x reference)
"""Trainium2 Bass kernel for CoreML alignment (duration -> frame expansion).

Math: each output frame f belongs to exactly one token l (the alignment
matrix is one-hot per frame: starts[l] <= f < cum_dur[l]), so

    en[:, c, f]  = d[:, tok(f), c]      (gather of d rows)
    asr[:, c, f] = t_en[:, c, tok(f)]   (gather of t_en columns)

and every column f >= T_a = sum(dur) is exactly zero.

Strategy (frame-sharded across 8 cores, no collectives):
  - Host computes cum_dur / frame->token indices (tiny metadata) and pads
    d / t_en^T with one zero row; inactive frames index the zero row.
  - The active frame range [0, T_a) is split across 8 cores, FS frames
    per core (multiple of 128). Each core:
      1. indirect-DMA gathers its frame-rows of d_pad and t_pad into SBUF,
         one 128-frame chunk per instruction (one index per partition),
      2. PE-transposes 128x128 blocks (bit-exact) into PSUM as each chunk
         lands,
      3. copies PSUM -> SBUF (split across Vector/Scalar engines) and DMAs
         [128, FS] slabs to its outputs.
  - Host places the 8 slabs into the full outputs; the tail past T_a
    stays zero (it is exactly zero mathematically).

Everything on device is data movement (DMA/transpose/copy) - results are
bit-exact fp32 copies of input values.
"""

import os
import numpy as np

MAX_FRAMES = 8192
N_CORES = 8
P = 128

# Results object of the last hardware run (exec_time_ns etc.), for the
# local test harness. The grader only uses the return value of kernel().
LAST_RESULTS = None


def _build_program(FS, NCHUNK, LP, C_D, C_T):
    import concourse.bacc as bacc
    import concourse.bass as bass
    import concourse.mybir as mybir
    import concourse.tile as tile

    nc = bacc.Bacc(
        "TRN2",
        target_bir_lowering=False,
        debug=False,
        num_devices=N_CORES,
    )

    C_W = C_D + C_T  # fused row: [d[l], t_en[:, l]]
    w_pad = nc.dram_tensor("w_pad", [LP, C_W], mybir.dt.float32, kind="ExternalInput")
    idx = nc.dram_tensor("idx", [P, NCHUNK], mybir.dt.int32, kind="ExternalInput")
    ident_in = nc.dram_tensor("ident", [P, P], mybir.dt.float32, kind="ExternalInput")
    en_out = nc.dram_tensor("en_out", [C_D, FS], mybir.dt.float32, kind="ExternalOutput")
    asr_out = nc.dram_tensor("asr_out", [C_T, FS], mybir.dt.float32, kind="ExternalOutput")

    MD = C_D // P  # en M-tiles (5)
    MT = C_T // P  # asr M-tiles (4)
    banks_per_tile = -(-(FS * 4) // 2048)  # PSUM bank = 2KB
    psum_bufs = max(1, 8 // banks_per_tile)

    with tile.TileContext(nc) as tc:
        with (
            tc.tile_pool(name="sbuf", bufs=1) as pool,
            tc.tile_pool(
                name="psum", bufs=psum_bufs, space=bass.MemorySpace.PSUM
            ) as psum_pool,
        ):
            # idx rides the (otherwise idle) Activation HWDGE ring so the
            # gather chain starts as early as possible.
            idx_sb = pool.tile([P, NCHUNK], mybir.dt.int32)
            nc.scalar.dma_start(out=idx_sb[:], in_=idx[:])
            ident = pool.tile([P, P], mybir.dt.float32)
            nc.sync.dma_start(out=ident[:], in_=ident_in[:])

            # Gather 128 fused frame-rows per instruction (one index per
            # partition): gath[p, :] = w_pad[idx[p, j], :] carries both the
            # d row and the t_en column for that frame's token.
            gath = []
            for j in range(NCHUNK):
                gw = pool.tile([P, C_W], mybir.dt.float32, tag=f"gw{j}")
                nc.gpsimd.indirect_dma_start(
                    out=gw[:],
                    out_offset=None,
                    in_=w_pad[:],
                    in_offset=bass.IndirectOffsetOnAxis(ap=idx_sb[:, j : j + 1], axis=0),
                )
                gath.append(gw)

            # PSUM accumulators: [chan 128, frame FS] per M-tile
            ps_en = [
                psum_pool.tile([P, FS], mybir.dt.float32, tag="ps", name=f"ps_en{m}")
                for m in range(MD)
            ]
            ps_asr = [
                psum_pool.tile([P, FS], mybir.dt.float32, tag="ps", name=f"ps_asr{m}")
                for m in range(MT)
            ]

            en_sb = pool.tile([P, MD, FS], mybir.dt.float32)
            asr_sb = pool.tile([P, MT, FS], mybir.dt.float32)

            # (psum tile, gather column offset, out SBUF slice, out DRAM slice)
            tiles = [
                (ps_en[m], m * P, en_sb[:, m, :], en_out[m * P : (m + 1) * P, :])
                for m in range(MD)
            ] + [
                (ps_asr[m], C_D + m * P, asr_sb[:, m, :], asr_out[m * P : (m + 1) * P, :])
                for m in range(MT)
            ]

            # Transpose each gathered [128 frames, 128 chans] block as its
            # chunk arrives; on the final chunk, interleave each M-tile's
            # last transpose with its copy + store so the tail is as short
            # as possible. Copies alternate Vector/Scalar; output DMAs
            # alternate the two HWDGE rings (SP / Activation).
            def copy_and_store(i, ps, sb, dram):
                if i % 2 == 0:
                    nc.vector.tensor_copy(sb, ps[:])
                    nc.sync.dma_start(out=dram, in_=sb)
                else:
                    nc.scalar.copy(sb, ps[:])
                    nc.scalar.dma_start(out=dram, in_=sb)

            for j in range(NCHUNK):
                last = j == NCHUNK - 1
                for i, (ps, col, sb, dram) in enumerate(tiles):
                    nc.tensor.transpose(
                        ps[:, j * P : (j + 1) * P],
                        gath[j][:, col : col + P],
                        ident[:],
                    )
                    if last:
                        copy_and_store(i, ps, sb, dram)

    nc.compile()
    return nc


def _prepare(pred_dur, d, t_en):
    """Host-side shard prep. Returns (FS, NCHUNK, in_maps) or None if T_a==0."""
    L = pred_dur.shape[1]
    C_D = d.shape[2]
    C_T = t_en.shape[1]

    dur = np.asarray(pred_dur[0], dtype=np.int64)
    cum = np.cumsum(dur)
    T_a = int(cum[-1])
    if T_a <= 0:
        return None

    FS = -(-T_a // N_CORES)  # ceil
    FS = -(-FS // P) * P  # round up to multiple of 128
    NCHUNK = FS // P

    # frame -> owning token; frames past T_a hit the zero row (index L)
    ind = np.searchsorted(cum, np.arange(T_a), side="right").astype(np.int32)
    idx_all = np.full(N_CORES * FS, L, dtype=np.int32)
    idx_all[:T_a] = ind

    # fused gather table: row l = [d[0, l, :], t_en[0, :, l]], plus a zero row
    w_pad = np.zeros((L + 1, C_D + C_T), np.float32)
    w_pad[:L, :C_D] = d[0]
    w_pad[:L, C_D:] = t_en[0].T
    ident = np.eye(P, dtype=np.float32)

    in_maps = []
    for k in range(N_CORES):
        # idx[p, j] = token index of frame k*FS + j*128 + p
        idx_k = np.ascontiguousarray(
            idx_all[k * FS : (k + 1) * FS].reshape(NCHUNK, P).T
        )
        in_maps.append({"w_pad": w_pad, "idx": idx_k, "ident": ident})
    return FS, NCHUNK, in_maps


def kernel(pred_dur, d, t_en):
    global LAST_RESULTS

    pred_dur = np.asarray(pred_dur)
    d = np.asarray(d, dtype=np.float32)
    t_en = np.asarray(t_en, dtype=np.float32)
    B, L = pred_dur.shape
    assert B == 1
    C_D = d.shape[2]
    C_T = t_en.shape[1]

    en = np.zeros((B, C_D, MAX_FRAMES), np.float32)
    asr = np.zeros((B, C_T, MAX_FRAMES), np.float32)

    prep = _prepare(pred_dur, d, t_en)
    if prep is None:
        return en, asr
    FS, NCHUNK, in_maps = prep

    from concourse.bass_utils import run_bass_kernel_spmd

    nc = _build_program(FS, NCHUNK, L + 1, C_D, C_T)
    trace = bool(os.environ.get("KERNEL_TRACE"))
    res = run_bass_kernel_spmd(
        nc,
        in_maps,
        core_ids=list(range(N_CORES)),
        trace=trace,
        trace_cores=list(range(N_CORES)) if trace else None,
    )
    LAST_RESULTS = res

    for k in range(N_CORES):
        f0 = k * FS
        f1 = min(f0 + FS, MAX_FRAMES)
        en[0, :, f0:f1] = res.results[k]["en_out"][:, : f1 - f0]
        asr[0, :, f0:f1] = res.results[k]["asr_out"][:, : f1 - f0]
    return en, asr


# revision 19
# speedup vs baseline: 1.0317x; 1.0317x over previous
"""Trainium2 Bass kernel for CoreML alignment (duration -> frame expansion).

Math: each output frame f belongs to exactly one token l (the alignment
matrix is one-hot per frame: starts[l] <= f < cum_dur[l]), so

    en[:, c, f]  = d[:, tok(f), c]      (gather of d rows)
    asr[:, c, f] = t_en[:, c, tok(f)]   (gather of t_en columns)

and every column f >= T_a = sum(dur) is exactly zero.

Strategy (frame-sharded across 8 cores, no collectives):
  - Host computes cum_dur / frame->token indices (tiny metadata) and pads
    d / t_en^T with one zero row; inactive frames index the zero row.
  - The active frame range [0, T_a) is split across 8 cores, FS frames
    per core (multiple of 128). Each core:
      1. indirect-DMA gathers its frame-rows of d_pad and t_pad into SBUF,
         one 128-frame chunk per instruction (one index per partition),
      2. PE-transposes 128x128 blocks (bit-exact) into PSUM as each chunk
         lands,
      3. copies PSUM -> SBUF (split across Vector/Scalar engines) and DMAs
         [128, FS] slabs to its outputs.
  - Host places the 8 slabs into the full outputs; the tail past T_a
    stays zero (it is exactly zero mathematically).

Everything on device is data movement (DMA/transpose/copy) - results are
bit-exact fp32 copies of input values.
"""

import os
import numpy as np

MAX_FRAMES = 8192
N_CORES = 8
P = 128

# Results object of the last hardware run (exec_time_ns etc.), for the
# local test harness. The grader only uses the return value of kernel().
LAST_RESULTS = None


def _build_program(FS, NCHUNK, LP, C_D, C_T):
    import concourse.bacc as bacc
    import concourse.bass as bass
    import concourse.mybir as mybir
    import concourse.tile as tile

    nc = bacc.Bacc(
        "TRN2",
        target_bir_lowering=False,
        debug=False,
        num_devices=N_CORES,
    )

    C_W = C_D + C_T  # fused row: [d[l], t_en[:, l]]
    w_pad = nc.dram_tensor("w_pad", [LP, C_W], mybir.dt.float32, kind="ExternalInput")
    idx = nc.dram_tensor("idx", [P, NCHUNK], mybir.dt.int32, kind="ExternalInput")
    ident_in = nc.dram_tensor("ident", [P, P], mybir.dt.float32, kind="ExternalInput")
    en_out = nc.dram_tensor("en_out", [C_D, FS], mybir.dt.float32, kind="ExternalOutput")
    asr_out = nc.dram_tensor("asr_out", [C_T, FS], mybir.dt.float32, kind="ExternalOutput")

    MD = C_D // P  # en M-tiles (5)
    MT = C_T // P  # asr M-tiles (4)
    banks_per_tile = -(-(FS * 4) // 2048)  # PSUM bank = 2KB
    psum_bufs = max(1, 8 // banks_per_tile)

    with tile.TileContext(nc) as tc:
        with (
            tc.tile_pool(name="sbuf", bufs=1) as pool,
            tc.tile_pool(
                name="psum", bufs=psum_bufs, space=bass.MemorySpace.PSUM
            ) as psum_pool,
        ):
            # idx rides the (otherwise idle) Activation HWDGE ring so the
            # gather chain starts as early as possible.
            idx_sb = pool.tile([P, NCHUNK], mybir.dt.int32)
            nc.scalar.dma_start(out=idx_sb[:], in_=idx[:])
            ident = pool.tile([P, P], mybir.dt.float32)
            nc.sync.dma_start(out=ident[:], in_=ident_in[:])

            # Gather 128 fused frame-rows per instruction (one index per
            # partition): gath[p, :] = w_pad[idx[p, j], :] carries both the
            # d row and the t_en column for that frame's token.
            gath = []
            for j in range(NCHUNK):
                gw = pool.tile([P, C_W], mybir.dt.float32, tag=f"gw{j}")
                nc.gpsimd.indirect_dma_start(
                    out=gw[:],
                    out_offset=None,
                    in_=w_pad[:],
                    in_offset=bass.IndirectOffsetOnAxis(ap=idx_sb[:, j : j + 1], axis=0),
                )
                gath.append(gw)

            en_sb = pool.tile([P, MD, FS], mybir.dt.float32)
            asr_sb = pool.tile([P, MT, FS], mybir.dt.float32)

            # (gather column offset, out SBUF view, out DRAM tensor)
            tiles = [(m * P, en_sb[:, m, :], en_out, m) for m in range(MD)] + [
                (C_D + m * P, asr_sb[:, m, :], asr_out, m) for m in range(MT)
            ]

            # Fully streamed: each [128 frames, 128 chans] block gets its own
            # PSUM bank -> transpose -> copy (Vector/Scalar alternating) ->
            # 64KB DMA out, rotated over three DMA rings (SP HWDGE,
            # Activation HWDGE, GPSIMD SWDGE). Output writes start as soon
            # as the first chunk is transposed instead of clustering at the
            # end of the kernel.
            n = 0
            for j in range(NCHUNK):
                jcol = slice(j * P, (j + 1) * P)
                for col, sb, dram, m in tiles:
                    ps = psum_pool.tile(
                        [P, P], mybir.dt.float32, tag="ps", name=f"ps{j}_{col}"
                    )
                    nc.tensor.transpose(ps[:], gath[j][:, col : col + P], ident[:])
                    if n % 2 == 0:
                        nc.vector.tensor_copy(sb[:, jcol], ps[:])
                    else:
                        nc.scalar.copy(sb[:, jcol], ps[:])
                    dma_eng = (nc.sync, nc.scalar, nc.gpsimd)[
                        2 if n % 4 == 3 else n % 2
                    ]
                    dma_eng.dma_start(
                        out=dram[m * P : (m + 1) * P, jcol], in_=sb[:, jcol]
                    )
                    n += 1

    nc.compile()
    return nc


def _prepare(pred_dur, d, t_en):
    """Host-side shard prep. Returns (FS, NCHUNK, in_maps) or None if T_a==0."""
    L = pred_dur.shape[1]
    C_D = d.shape[2]
    C_T = t_en.shape[1]

    dur = np.asarray(pred_dur[0], dtype=np.int64)
    cum = np.cumsum(dur)
    T_a = int(cum[-1])
    if T_a <= 0:
        return None

    FS = -(-T_a // N_CORES)  # ceil
    FS = -(-FS // P) * P  # round up to multiple of 128
    NCHUNK = FS // P

    # frame -> owning token; frames past T_a hit the zero row (index L)
    ind = np.searchsorted(cum, np.arange(T_a), side="right").astype(np.int32)
    idx_all = np.full(N_CORES * FS, L, dtype=np.int32)
    idx_all[:T_a] = ind

    # fused gather table: row l = [d[0, l, :], t_en[0, :, l]], plus a zero row
    w_pad = np.zeros((L + 1, C_D + C_T), np.float32)
    w_pad[:L, :C_D] = d[0]
    w_pad[:L, C_D:] = t_en[0].T
    ident = np.eye(P, dtype=np.float32)

    in_maps = []
    for k in range(N_CORES):
        # idx[p, j] = token index of frame k*FS + j*128 + p
        idx_k = np.ascontiguousarray(
            idx_all[k * FS : (k + 1) * FS].reshape(NCHUNK, P).T
        )
        in_maps.append({"w_pad": w_pad, "idx": idx_k, "ident": ident})
    return FS, NCHUNK, in_maps


def kernel(pred_dur, d, t_en):
    global LAST_RESULTS

    pred_dur = np.asarray(pred_dur)
    d = np.asarray(d, dtype=np.float32)
    t_en = np.asarray(t_en, dtype=np.float32)
    B, L = pred_dur.shape
    assert B == 1
    C_D = d.shape[2]
    C_T = t_en.shape[1]

    en = np.zeros((B, C_D, MAX_FRAMES), np.float32)
    asr = np.zeros((B, C_T, MAX_FRAMES), np.float32)

    prep = _prepare(pred_dur, d, t_en)
    if prep is None:
        return en, asr
    FS, NCHUNK, in_maps = prep

    from concourse.bass_utils import run_bass_kernel_spmd

    nc = _build_program(FS, NCHUNK, L + 1, C_D, C_T)
    trace = bool(os.environ.get("KERNEL_TRACE"))
    res = run_bass_kernel_spmd(
        nc,
        in_maps,
        core_ids=list(range(N_CORES)),
        trace=trace,
        trace_cores=list(range(N_CORES)) if trace else None,
    )
    LAST_RESULTS = res

    for k in range(N_CORES):
        f0 = k * FS
        f1 = min(f0 + FS, MAX_FRAMES)
        en[0, :, f0:f1] = res.results[k]["en_out"][:, : f1 - f0]
        asr[0, :, f0:f1] = res.results[k]["asr_out"][:, : f1 - f0]
    return en, asr


# revision 24
# speedup vs baseline: 1.0552x; 1.0228x over previous
"""Trainium2 Bass kernel for CoreML alignment (duration -> frame expansion).

Math: each output frame f belongs to exactly one token l (the alignment
matrix is one-hot per frame: starts[l] <= f < cum_dur[l]), so

    en[:, c, f]  = d[:, tok(f), c]      (gather of d rows)
    asr[:, c, f] = t_en[:, c, tok(f)]   (gather of t_en columns)

and every column f >= T_a = sum(dur) is exactly zero.

Strategy (frame-sharded across 8 cores, no collectives):
  - Host computes cum_dur / frame->token indices (tiny metadata) and pads
    d / t_en^T with one zero row; inactive frames index the zero row.
  - The active frame range [0, T_a) is split across 8 cores, FS frames
    per core (multiple of 128). Each core:
      1. indirect-DMA gathers its frame-rows of d_pad and t_pad into SBUF,
         one 128-frame chunk per instruction (one index per partition),
      2. PE-transposes 128x128 blocks (bit-exact) into PSUM as each chunk
         lands,
      3. copies PSUM -> SBUF (split across Vector/Scalar engines) and DMAs
         [128, FS] slabs to its outputs.
  - Host places the 8 slabs into the full outputs; the tail past T_a
    stays zero (it is exactly zero mathematically).

Everything on device is data movement (DMA/transpose/copy) - results are
bit-exact fp32 copies of input values.
"""

import os
import numpy as np

MAX_FRAMES = 8192
N_CORES = 8
P = 128

# Results object of the last hardware run (exec_time_ns etc.), for the
# local test harness. The grader only uses the return value of kernel().
LAST_RESULTS = None


def _build_program(FS, NCHUNK, LP, C_D, C_T):
    import concourse.bacc as bacc
    import concourse.bass as bass
    import concourse.mybir as mybir
    import concourse.tile as tile

    nc = bacc.Bacc(
        "TRN2",
        target_bir_lowering=False,
        debug=False,
        num_devices=N_CORES,
    )

    C_W = C_D + C_T  # fused row: [d[l], t_en[:, l]]
    w_pad = nc.dram_tensor("w_pad", [LP, C_W], mybir.dt.float32, kind="ExternalInput")
    d_pad = nc.dram_tensor("d_pad", [LP, C_D], mybir.dt.float32, kind="ExternalInput")
    t_pad = nc.dram_tensor("t_pad", [LP, C_T], mybir.dt.float32, kind="ExternalInput")
    idx = nc.dram_tensor("idx", [P, NCHUNK], mybir.dt.int32, kind="ExternalInput")
    ident_in = nc.dram_tensor("ident", [P, P], mybir.dt.float32, kind="ExternalInput")
    en_out = nc.dram_tensor("en_out", [C_D, FS], mybir.dt.float32, kind="ExternalOutput")
    asr_out = nc.dram_tensor("asr_out", [C_T, FS], mybir.dt.float32, kind="ExternalOutput")

    MD = C_D // P  # en M-tiles (5)
    MT = C_T // P  # asr M-tiles (4)
    banks_per_tile = -(-(FS * 4) // 2048)  # PSUM bank = 2KB
    psum_bufs = max(1, 8 // banks_per_tile)

    with tile.TileContext(nc) as tc:
        with (
            tc.tile_pool(name="sbuf", bufs=1) as pool,
            tc.tile_pool(
                name="psum", bufs=psum_bufs, space=bass.MemorySpace.PSUM
            ) as psum_pool,
        ):
            # idx rides the (otherwise idle) Activation HWDGE ring so the
            # gather chain starts as early as possible.
            idx_sb = pool.tile([P, NCHUNK], mybir.dt.int32)
            nc.scalar.dma_start(out=idx_sb[:], in_=idx[:])
            ident = pool.tile([P, P], mybir.dt.float32)
            nc.sync.dma_start(out=ident[:], in_=ident_in[:])

            # Gather 128 fused frame-rows per instruction (one index per
            # partition): gath[p, :] = w_pad[idx[p, j], :] carries both the
            # d row and the t_en column for that frame's token. The LAST
            # chunk is gathered from the split d/t tables instead, so the
            # en tiles complete one gather earlier than asr tiles and the
            # output stores stagger instead of clustering.
            def gather(j, src, width, tag):
                g = pool.tile([P, width], mybir.dt.float32, tag=tag, name=tag)
                nc.gpsimd.indirect_dma_start(
                    out=g[:],
                    out_offset=None,
                    in_=src[:],
                    in_offset=bass.IndirectOffsetOnAxis(ap=idx_sb[:, j : j + 1], axis=0),
                )
                return g

            gath = [gather(j, w_pad, C_W, f"gw{j}") for j in range(NCHUNK - 1)]
            gd_last = gather(NCHUNK - 1, d_pad, C_D, "gd_last")
            gt_last = gather(NCHUNK - 1, t_pad, C_T, "gt_last")

            # PSUM accumulators: [chan 128, frame FS] per M-tile. 9 tiles on
            # 8 banks: the last-allocated (asr m=3) waits for the first en
            # copy to free a bank, by which time its gather data is present.
            ps_en = [
                psum_pool.tile([P, FS], mybir.dt.float32, tag="ps", name=f"ps_en{m}")
                for m in range(MD)
            ]
            ps_asr = [
                psum_pool.tile([P, FS], mybir.dt.float32, tag="ps", name=f"ps_asr{m}")
                for m in range(MT)
            ]
            en_sb = pool.tile([P, MD, FS], mybir.dt.float32)
            asr_sb = pool.tile([P, MT, FS], mybir.dt.float32)

            # chunks 0..NCHUNK-2 from the fused gathers
            for j in range(NCHUNK - 1):
                for m in range(MD):
                    nc.tensor.transpose(
                        ps_en[m][:, j * P : (j + 1) * P],
                        gath[j][:, m * P : (m + 1) * P],
                        ident[:],
                    )
                for m in range(MT):
                    nc.tensor.transpose(
                        ps_asr[m][:, j * P : (j + 1) * P],
                        gath[j][:, C_D + m * P : C_D + (m + 1) * P],
                        ident[:],
                    )

            # final chunk: per M-tile transpose -> copy -> store, en first
            # (its gather lands one instruction earlier than asr's)
            jcol = slice((NCHUNK - 1) * P, NCHUNK * P)

            def finish(ps, g, gcol, sb, dram, m, copy_eng, dma_eng):
                nc.tensor.transpose(ps[:, jcol], g[:, gcol : gcol + P], ident[:])
                if copy_eng is nc.vector:
                    nc.vector.tensor_copy(sb[:, m, :], ps[:])
                else:
                    nc.scalar.copy(sb[:, m, :], ps[:])
                dma_eng.dma_start(out=dram[m * P : (m + 1) * P, :], in_=sb[:, m, :])

            for m in range(MD):
                finish(
                    ps_en[m], gd_last, m * P, en_sb, en_out, m,
                    nc.vector if m % 2 == 0 else nc.scalar,
                    nc.sync if m % 2 == 0 else nc.scalar,
                )
            for m in range(MT):
                if m == MT - 1:
                    dma_eng = nc.gpsimd  # Pool ring is idle after the gathers
                elif m % 2 == 0:
                    dma_eng = nc.scalar
                else:
                    dma_eng = nc.sync
                finish(
                    ps_asr[m], gt_last, m * P, asr_sb, asr_out, m,
                    nc.scalar if m % 2 == 0 else nc.vector,
                    dma_eng,
                )

    nc.compile()
    return nc


def _prepare(pred_dur, d, t_en):
    """Host-side shard prep. Returns (FS, NCHUNK, in_maps) or None if T_a==0."""
    L = pred_dur.shape[1]
    C_D = d.shape[2]
    C_T = t_en.shape[1]

    dur = np.asarray(pred_dur[0], dtype=np.int64)
    cum = np.cumsum(dur)
    T_a = int(cum[-1])
    if T_a <= 0:
        return None

    FS = -(-T_a // N_CORES)  # ceil
    FS = -(-FS // P) * P  # round up to multiple of 128
    NCHUNK = FS // P

    # frame -> owning token; frames past T_a hit the zero row (index L)
    ind = np.searchsorted(cum, np.arange(T_a), side="right").astype(np.int32)
    idx_all = np.full(N_CORES * FS, L, dtype=np.int32)
    idx_all[:T_a] = ind

    # fused gather table: row l = [d[0, l, :], t_en[0, :, l]], plus a zero
    # row; split tables for the final chunk's gathers
    w_pad = np.zeros((L + 1, C_D + C_T), np.float32)
    w_pad[:L, :C_D] = d[0]
    w_pad[:L, C_D:] = t_en[0].T
    d_pad = np.ascontiguousarray(w_pad[:, :C_D])
    t_pad = np.ascontiguousarray(w_pad[:, C_D:])
    ident = np.eye(P, dtype=np.float32)

    in_maps = []
    for k in range(N_CORES):
        # idx[p, j] = token index of frame k*FS + j*128 + p
        idx_k = np.ascontiguousarray(
            idx_all[k * FS : (k + 1) * FS].reshape(NCHUNK, P).T
        )
        in_maps.append(
            {"w_pad": w_pad, "d_pad": d_pad, "t_pad": t_pad, "idx": idx_k, "ident": ident}
        )
    return FS, NCHUNK, in_maps


def kernel(pred_dur, d, t_en):
    global LAST_RESULTS

    pred_dur = np.asarray(pred_dur)
    d = np.asarray(d, dtype=np.float32)
    t_en = np.asarray(t_en, dtype=np.float32)
    B, L = pred_dur.shape
    assert B == 1
    C_D = d.shape[2]
    C_T = t_en.shape[1]

    en = np.zeros((B, C_D, MAX_FRAMES), np.float32)
    asr = np.zeros((B, C_T, MAX_FRAMES), np.float32)

    prep = _prepare(pred_dur, d, t_en)
    if prep is None:
        return en, asr
    FS, NCHUNK, in_maps = prep

    from concourse.bass_utils import run_bass_kernel_spmd

    nc = _build_program(FS, NCHUNK, L + 1, C_D, C_T)
    trace = bool(os.environ.get("KERNEL_TRACE"))
    res = run_bass_kernel_spmd(
        nc,
        in_maps,
        core_ids=list(range(N_CORES)),
        trace=trace,
        trace_cores=list(range(N_CORES)) if trace else None,
    )
    LAST_RESULTS = res

    for k in range(N_CORES):
        f0 = k * FS
        f1 = min(f0 + FS, MAX_FRAMES)
        en[0, :, f0:f1] = res.results[k]["en_out"][:, : f1 - f0]
        asr[0, :, f0:f1] = res.results[k]["asr_out"][:, : f1 - f0]
    return en, asr
